# revision 1
# baseline (speedup 1.0000x reference)
"""Multi-head attention (nn_MultiHeadDotProductAttention) for 8 Trainium2 cores.

Sharding: core c -> (batch = c // 2, head-group = c % 2 of 8 heads).
Each core computes a partial output [T, E] (its 8 heads' contribution to the
output projection); the host sums the two partials per batch. No collectives.

Per-core pipeline (bf16 matmuls, fp32 PSUM accumulation):
  1. SWDGE-cast X to bf16 DRAM staging; xbar DMA-transpose -> XT [emb, tok].
  2. Projections: kT/qT pair-packed [128=(2 heads x 64d), tok]; v stored as
     [v_even | ones | v_odd] per pair so each head's AV lhsT is one
     contiguous [128,128] slice (ones columns produce softmax denominators).
  3. Attention per head-pair, q-block 1024, k-chunk 128:
       S^T = kT.T @ qT  (both heads in PE row groups 0-63/64-127)
       expP^T = exp(S^T) (ScalarE, 2-bank PSUM [128,1024] -> bf16 SBUF)
       out^T accum += [v|ones].T @ expP^T  (attn rows + replicated sum rows)
     normalize: 1/s = exp(-ln(s)) on ScalarE -> K=1 PE broadcast matmul ->
     DVE multiply -> attnT.  qT-projection of the next pair fills PE gaps.
  4. Output projection: out += attnT.T @ Wo, contracting (h,d) in pair chunks.
"""

import numpy as np

import concourse.bass as bass
import concourse.mybir as mybir
import concourse.tile as tile
from concourse.bass_utils import run_bass_kernel_spmd
from concourse.masks import make_identity

F32 = mybir.dt.float32
F32R = mybir.dt.float32r
P = 128


# ---------------------------------------------------------------------------
# Workaround: this walrus build rejects Drain instructions carrying more than
# one sync wait ("Too many sync wait commands", setupSyncWait<CTRL_NO_STRUCT>).
# Emit the tail-drain waits as individual single-wait instructions instead.
def _patched_drain_and_barrier(self, tick_clock, wait_clock):
    from bass_rust import ScopedClock

    dummy = mybir.InstNoOp(
        name=f"drain_wait_harvest_{self.nc.next_id()}", ins=[], outs=[]
    )
    dummy.engine = mybir.EngineType.SP
    wait_clock.add_sem_waits(dummy, ScopedClock({None: tick_clock.global_clock}))

    waits = []
    if dummy.sync_info is not None and dummy.sync_info.on_wait:
        waits = list(dummy.sync_info.on_wait)

    assert self.sems is not None
    sems_by_name = {h.name: h for h in self.sems.allocated().values()}
    for w in waits:
        sem = sems_by_name[w.ant_name]
        assert w.wait_mode in ("sem-ge-imm", "sem-ge"), w.wait_mode
        self.nc.sync.wait_ge(sem, w.wait_value)

    self.nc.sync.drain()
    self.nc.all_engine_barrier()
    popped = self.nc._tile_sem_poison_stack.pop()
    assert popped is self._sem_poison
    self.nc.clear_and_free_semaphores(list(self.sems.allocated().values()))
    self.nc.all_engine_barrier()


tile.TileContext._drain_and_barrier = _patched_drain_and_barrier


def _split_multi_waits(nc):
    """Same walrus limitation: any instruction with >1 sync wait fails codegen.
    Move all but one wait onto single-wait NoOps inserted just before the
    instruction (same engine, same basic block) — semantically identical since
    Tile semaphores are monotonic within the kernel."""
    n = 0
    for bb in nc.m.functions[0].blocks:
        insts = bb.instructions
        out = []
        changed = False
        for inst in insts:
            si = inst.sync_info
            if si is not None and si.on_wait and len(si.on_wait) > 1:
                waits = list(si.on_wait)
                for w in waits[:-1]:
                    nop = mybir.InstNoOp(name=f"{inst.name}_sw{n}", ins=[], outs=[])
                    n += 1
                    nop.engine = inst.engine
                    nop.sync_info = mybir.SyncInfo(on_wait=[w], on_update=[])
                    out.append(nop)
                inst.sync_info = mybir.SyncInfo(
                    on_wait=[waits[-1]], on_update=list(si.on_update or [])
                )
                changed = True
            out.append(inst)
        if changed:
            bb.instructions = out
# ---------------------------------------------------------------------------


def build_attention_nc(T, KV, E, HL, D=64):
    """Bass module for one core: xq [T,E], xkv [KV,E], wq/wk/wv [E,HL,D]
    (wq pre-scaled by 1/sqrt(D) on host), wo [HL,D,E] -> out [T,E] partial.

    bf16 matmul pipeline (fp32 PSUM accumulation). X transposed via bf16
    DMA-transpose (xbar), not the PE. One shared 8-bank PSUM pool with tags
    s0/s1/av0/av1 reused across phases so projection and attention overlap.
    """
    assert D == 64 and HL % 2 == 0 and E % P == 0 and T % 1024 == 0 and KV % P == 0
    BF16 = mybir.dt.bfloat16
    NEC = E // P          # emb 128-chunks
    PAIRS = HL // 2
    NKC = KV // P         # k chunks of 128
    HD = HL * D           # local head-dim product
    QB = 1024             # attention q-block

    nc = bass.Bass("TRN2", target_bir_lowering=False, debug=False)

    def mm_wide(out_ps, lhsT, rhs, start, stop, width=512):
        """matmul with moving dim wider than one PSUM bank: slice into
        512-wide column chunks (PSUM out is capped at one bank)."""
        n = rhs.shape[-1]
        for c0 in range(0, n, width):
            c1 = min(n, c0 + width)
            nc.tensor.matmul(
                out_ps[:, c0:c1], lhsT, rhs[:, c0:c1], start=start, stop=stop
            )

    xq_d = nc.dram_tensor("xq", [T, E], F32, kind="ExternalInput").ap()
    xkv_d = nc.dram_tensor("xkv", [KV, E], F32, kind="ExternalInput").ap()
    wq_d = nc.dram_tensor("wq", [E, HL, D], F32, kind="ExternalInput").ap()
    wk_d = nc.dram_tensor("wk", [E, HL, D], F32, kind="ExternalInput").ap()
    wv_d = nc.dram_tensor("wv", [E, HL, D], F32, kind="ExternalInput").ap()
    wo_d = nc.dram_tensor("wo", [HL, D, E], F32, kind="ExternalInput").ap()
    out_d = nc.dram_tensor("out", [T, E], F32, kind="ExternalOutput").ap()
    xq_bf = nc.dram_tensor("xq_bf", [T, E], BF16).ap()
    xkv_bf = nc.dram_tensor("xkv_bf", [KV, E], BF16).ap()

    with tile.TileContext(nc) as tc:
        with (
            tc.tile_pool(name="const", bufs=1) as constp,
            tc.tile_pool(name="persist", bufs=1) as persist,
            tc.tile_pool(name="wpool", bufs=2) as wpool,
            tc.tile_pool(name="wopool", bufs=1) as wopool,
            tc.tile_pool(name="xpool", bufs=3) as xpool,
            tc.tile_pool(name="epool", bufs=2) as epool,
            tc.tile_pool(name="rpool", bufs=2) as rpool,
            tc.tile_pool(name="ostage", bufs=3) as ostage,
            tc.tile_pool(name="psum", bufs=1, space="PSUM") as psum,
        ):
            # cast the activations to bf16 DRAM staging (SWDGE cast), one
            # DMA per 1024-row half so the first transposes start early;
            # DMA-transposes then stream straight from DRAM.
            for r0 in range(0, KV, 1024):
                nc.gpsimd.dma_start(
                    xkv_bf[r0 : r0 + 1024, :], xkv_d[r0 : r0 + 1024, :]
                )
            for r0 in range(0, T, 1024):
                nc.gpsimd.dma_start(
                    xq_bf[r0 : r0 + 1024, :], xq_d[r0 : r0 + 1024, :]
                )

            ones32 = constp.tile([P, P], F32, tag="ones32")
            nc.vector.memset(ones32[:], 1.0)
            ones_sb = constp.tile([P, P], BF16, tag="ones")
            nc.vector.tensor_copy(out=ones_sb[:], in_=ones32[:])

            kT_sb = persist.tile([P, PAIRS, KV], BF16, tag="kT")
            qT_sb = persist.tile([P, PAIRS, T], BF16, tag="qT")
            attnT_sb = persist.tile([P, PAIRS, T], BF16, tag="attnT")
            # v layout per (kc, pair): [v_even(64) | ones(64) | v_odd(64)] so
            # each head's AV lhsT is a contiguous [128, 128] slice:
            #   even head -> cols 0:128  (attn rows 0-63,  sum rows 64-127)
            #   odd head  -> cols 64:192 (sum rows 0-63,   attn rows 64-127)
            v_sb = persist.tile([P, NKC, PAIRS, 3, D], BF16, tag="v")
            nc.vector.tensor_copy(
                out=v_sb[:, :, :, 1, :],
                in_=ones_sb[:, None, None, :D].to_broadcast((P, NKC, PAIRS, D)),
            )

            wk_sb = wpool.tile([P, NEC, HD], BF16, tag="wslot")
            nc.gpsimd.dma_start(
                wk_sb[:], wk_d.rearrange("(ec p) h d -> p ec (h d)", p=P)
            )
            wv_sb = wpool.tile([P, NEC, HD], BF16, tag="wslot")
            nc.gpsimd.dma_start(
                wv_sb[:], wv_d.rearrange("(ec p) h d -> p ec (h d)", p=P)
            )
            wo_sb = wopool.tile([P, PAIRS, E], BF16, tag="wo")
            nc.gpsimd.dma_start(
                wo_sb[:],
                wo_d.rearrange("(pr two) d e -> (two d) pr e", two=2),
            )

            # round-robin psum tag allocator for projection tiles
            _ptag = [0]
            PS_TAGS = ["s0", "s1", "av0", "av1"]

            def proj_psum():
                t = PS_TAGS[_ptag[0] % 4]
                _ptag[0] += 1
                return psum.tile([P, QB], F32, tag=t, name=f"pp_{t}")

            def transpose_half(xbf_d, th, n_tt):
                """DMA-transpose token rows [th*1024, ...) of the bf16
                staging copy into xT [P, NEC, n_tt*128]."""
                xT = xpool.tile([P, NEC, 1024], BF16, tag="xT")
                for tt in range(n_tt):
                    t0 = th * 1024 + tt * P
                    nc.sync.dma_start_transpose(
                        xT[:, :, tt * P : (tt + 1) * P], xbf_d[t0 : t0 + P, :]
                    )
                return xT

            # ---- kv path: kT pairs + v ----
            for th in range((KV + 1023) // 1024):
                n_tt = min(8, (KV - th * 1024) // P)
                xT = transpose_half(xkv_bf, th, n_tt)
                for pr in range(PAIRS):
                    for tq in range((n_tt * P) // QB):
                        ps = proj_psum()
                        for ec in range(NEC):
                            mm_wide(
                                ps,
                                wk_sb[:, ec, pr * P : (pr + 1) * P],
                                xT[:, ec, tq * QB : (tq + 1) * QB],
                                start=(ec == 0),
                                stop=(ec == NEC - 1),
                            )
                        nc.vector.tensor_copy(
                            out=kT_sb[
                                :, pr, th * 1024 + tq * QB : th * 1024 + (tq + 1) * QB
                            ],
                            in_=ps[:],
                        )
                for kc in range(n_tt):
                    ps = proj_psum()
                    for ec in range(NEC):
                        nc.tensor.matmul(
                            ps[:, :HD],
                            xT[:, ec, kc * P : (kc + 1) * P],
                            wv_sb[:, ec, :],
                            start=(ec == 0),
                            stop=(ec == NEC - 1),
                        )
                    nc.vector.tensor_copy(
                        out=v_sb[:, th * 8 + kc, :, 0::2, :],
                        in_=ps[:, :HD].rearrange(
                            "p (pr two d) -> p pr two d", two=2, d=D
                        ),
                    )

            # ---- q path: transposes for all halves, then per-pair
            # qT-projection interleaved with that pair's attention so the
            # PE fills ACT-bound attention gaps with projection work ----
            wq_sb = wpool.tile([P, NEC, HD], BF16, tag="wslot")
            nc.gpsimd.dma_start(
                wq_sb[:], wq_d.rearrange("(ec p) h d -> p ec (h d)", p=P)
            )
            n_th = (T + 1023) // 1024
            qxT = [
                transpose_half(xq_bf, th, min(8, (T - th * 1024) // P))
                for th in range(n_th)
            ]

            for pr in range(PAIRS):
                for th in range(n_th):
                    n_tt = min(8, (T - th * 1024) // P)
                    for tq in range((n_tt * P) // QB):
                        ps = proj_psum()
                        for ec in range(NEC):
                            mm_wide(
                                ps,
                                wq_sb[:, ec, pr * P : (pr + 1) * P],
                                qxT[th][:, ec, tq * QB : (tq + 1) * QB],
                                start=(ec == 0),
                                stop=(ec == NEC - 1),
                            )
                        nc.vector.tensor_copy(
                            out=qT_sb[
                                :, pr, th * 1024 + tq * QB : th * 1024 + (tq + 1) * QB
                            ],
                            in_=ps[:],
                        )
                # ---- attention for this pair ----
                for qhalf in range(T // QB):
                    q0 = qhalf * QB
                    av = {}
                    for j in range(2):
                        av[j] = psum.tile(
                            [P, QB], F32, tag=f"av{j}", name=f"av{j}"
                        )
                    for kc in range(NKC):
                        s_ps = {}
                        for j in range(2):
                            s_ps[j] = psum.tile(
                                [P, QB], F32, tag=f"s{j}", name=f"s{j}"
                            )
                            mm_wide(
                                s_ps[j],
                                kT_sb[
                                    j * D : (j + 1) * D, pr, kc * P : (kc + 1) * P
                                ],
                                qT_sb[j * D : (j + 1) * D, pr, q0 : q0 + QB],
                                start=True,
                                stop=True,
                            )
                        eps = {}
                        for j in range(2):
                            eps[j] = epool.tile([P, QB], BF16, tag=f"e{j}", name=f"e{j}")
                            nc.scalar.activation(
                                eps[j][:],
                                s_ps[j][:],
                                mybir.ActivationFunctionType.Exp,
                            )
                        for j in range(2):
                            mm_wide(
                                av[j],
                                v_sb[:, kc, pr, :, :]
                                .rearrange("p t d -> p (t d)")[:, j * D : j * D + P],
                                eps[j][:],
                                start=(kc == 0),
                                stop=(kc == NKC - 1),
                            )
                    # normalize + write attnT
                    for j in range(2):
                        arow = 0 if j == 0 else D  # attention rows
                        srow = D if j == 0 else 0  # sum rows
                        # reciprocal of the sums on ACT: 1/s = exp(-ln(s));
                        # both functions live in the natural_log_exp table
                        # set, so no table switching with the softmax exps.
                        lns = rpool.tile([P, QB], F32, tag="lns")
                        nc.scalar.activation(
                            lns[srow : srow + 1, :],
                            av[j][srow : srow + 1, :],
                            mybir.ActivationFunctionType.Ln,
                        )
                        rt = rpool.tile([P, QB], BF16, tag="r")
                        nc.scalar.activation(
                            rt[srow : srow + 1, :],
                            lns[srow : srow + 1, :],
                            mybir.ActivationFunctionType.Exp,
                            scale=-1.0,
                        )
                        bc = psum.tile([P, QB], F32, tag=f"s{j}", name=f"bc{j}")
                        mm_wide(
                            bc,
                            ones_sb[srow : srow + 1, :],
                            rt[srow : srow + 1, :],
                            start=True,
                            stop=True,
                        )
                        bcs = rpool.tile([P, QB], F32, tag="bcs")
                        nc.vector.tensor_copy(
                            out=bcs[arow : arow + D, :], in_=bc[arow : arow + D, :]
                        )
                        nc.vector.tensor_tensor(
                            attnT_sb[arow : arow + D, pr, q0 : q0 + QB],
                            av[j][arow : arow + D, :],
                            bcs[arow : arow + D, :],
                            mybir.AluOpType.mult,
                        )
            # ---- output projection over all tokens ----
            for tt in range(T // P):
                stage = ostage.tile([P, E], F32, tag="ostage")
                ops = psum.tile(
                    [P, QB], F32, tag=f"av{tt % 2}", name=f"oproj{tt % 2}"
                )
                for pr in range(PAIRS):
                    mm_wide(
                        ops[:, :E],
                        attnT_sb[:, pr, tt * P : (tt + 1) * P],
                        wo_sb[:, pr, :],
                        start=(pr == 0),
                        stop=(pr == PAIRS - 1),
                    )
                nc.vector.tensor_copy(out=stage[:], in_=ops[:, :E])
                nc.sync.dma_start(out_d[tt * P : (tt + 1) * P, :], stage[:])

    _split_multi_waits(nc)
    return nc


_NC_CACHE = {}


def _get_nc(key):
    if key not in _NC_CACHE:
        _NC_CACHE[key] = build_attention_nc(*key)
    return _NC_CACHE[key]


def kernel(inputs_q, inputs_kv, Wq, Wk, Wv, Wo, _trace=False):
    inputs_q = np.ascontiguousarray(np.asarray(inputs_q), dtype=np.float32)
    inputs_kv = np.ascontiguousarray(np.asarray(inputs_kv), dtype=np.float32)
    Wq = np.asarray(Wq, dtype=np.float32)
    Wk = np.asarray(Wk, dtype=np.float32)
    Wv = np.asarray(Wv, dtype=np.float32)
    Wo = np.asarray(Wo, dtype=np.float32)

    B, T, E = inputs_q.shape
    KV = inputs_kv.shape[1]
    H, D = Wq.shape[1], Wq.shape[2]
    HL = H // 2  # heads per core (head axis split 2-way)
    depth_scale = np.float32(np.sqrt(np.float32(D)))
    Wq_s = (Wq / depth_scale).astype(np.float32)

    nc = _get_nc((T, KV, E, HL, D))

    in_maps = []
    for c in range(8):
        bi, hg = c // 2, c % 2
        hs = slice(hg * HL, (hg + 1) * HL)
        in_maps.append(
            {
                "xq": inputs_q[bi],
                "xkv": inputs_kv[bi],
                "wq": np.ascontiguousarray(Wq_s[:, hs, :]),
                "wk": np.ascontiguousarray(Wk[:, hs, :]),
                "wv": np.ascontiguousarray(Wv[:, hs, :]),
                "wo": np.ascontiguousarray(Wo[hs]),
            }
        )

    res = run_bass_kernel_spmd(nc, in_maps, core_ids=list(range(8)), trace=_trace)

    out = np.empty((B, T, E), dtype=np.float32)
    for bi in range(B):
        out[bi] = res.results[2 * bi]["out"] + res.results[2 * bi + 1]["out"]
    if _trace:
        kernel._last_results = res
    return out

